# revision 6
# baseline (speedup 1.0000x reference)
"""Trainium2 Bass kernel for nn_EnsembleModel (ensemble recommender).

Contract: kernel(**inputs) takes FULL unsharded inputs (as produced by the
reference setup_inputs) and returns the FULL [512, 20] int32 output.

Strategy (8 NeuronCores, SPMD — identical program, per-core data):
  - items sharded 8x: each core owns 6250 catalog columns of user_ratings
    (padded to 6656 = 13*512) and computes k_preds = softmax(X@U.T/sqrt(32)) @ R
    for its shard with exact-fp32 PE matmuls, then extracts per-row top-40
    (values + indices) with max8/max_index/match_replace.
  - the two decoder branches are column-sharded 8x as well (64 / 256 cols per
    core); each core computes its slice of preds = (X@W_prior)@W_dec_slice,
    applies the gathered mask columns, and extracts per-row top-40.
  - host merges the per-core candidate lists (provably lossless: a shard-local
    top-40 always contains the shard's contribution to the global top-40) and
    reproduces the reference's fused scatter-add + final top-20 in float32.
"""

import numpy as np

_B, _D, _LAT = 512, 32, 128
_NS, _NM, _NI, _NU = 500, 2000, 50000, 2000
_NC = 8
_SHW = _NI // _NC            # 6250 items per core
_CH = 512
_NCH = 13
_SHP = _CH * _NCH            # 6656 padded shard width
_SSL = 64                    # s-branch cols per core  (8*64  >= 500)
_MSL = 256                   # m-branch cols per core  (8*256 >= 2000)
_TK = 40                     # two_k
_K = 20

_cache = {}


def _build_program():
    import concourse.bacc as bacc
    import concourse.tile as tile
    from concourse import mybir

    nc = bacc.Bacc("TRN2", target_bir_lowering=False, debug=False, num_devices=_NC)
    f32 = mybir.dt.float32
    f32r = mybir.dt.float32r
    u32 = mybir.dt.uint32

    ins = {}
    def inp(name, shape):
        ins[name] = nc.dram_tensor(name, shape, f32, kind="ExternalInput").ap()
    inp("XT", [_D, _B])            # X transposed (host-prepped)
    inp("UT", [_D, _NU])           # user_personalities transposed
    inp("RH", [_NU, _SHP])         # ratings shard hi (11-bit mantissa)
    inp("RL", [_NU, _SHP])         # ratings shard lo (R - RH, exact)
    inp("Wsp", [_D, _LAT])         # W_sprior
    inp("Wmp", [_D, _LAT])         # W_mprior
    inp("Wsd", [_LAT, _SSL])       # W_sdec column slice (zero-padded)
    inp("Wmd", [_LAT, _MSL])       # W_mdec column slice
    inp("MS", [_B, _SSL])          # mask cols for the s slice
    inp("MM", [_B, _MSL])          # mask cols for the m slice
    inp("EYE", [128, 128])         # identity for PE transpose

    outs = {}
    def outp(name, shape, dt):
        outs[name] = nc.dram_tensor(name, shape, dt, kind="ExternalOutput").ap()
    outp("KV", [_B, _NCH * 16], f32)
    outp("KI", [_B, _NCH * 16], u32)
    outp("SV", [_B, _TK], f32)
    outp("SI", [_B, _TK], u32)
    outp("MV", [_B, _TK], f32)
    outp("MI", [_B, _TK], u32)

    RT = 4                       # row tiles of 128
    UCW = 500                    # logits chunk width (4 * 500 = 2000)
    UC = _NU // UCW
    KTS = [(o, min(128, _NU - o)) for o in range(0, _NU, 128)]  # 15x128 + 1x80
    KT = len(KTS)
    inv_scale = float(np.float32(1.0) / np.float32(np.sqrt(np.float32(_D))))

    with tile.TileContext(nc) as tc:
        with tc.tile_pool(name="persist", bufs=1) as per:
            xt = per.tile([_D, _B], f32, name="xt")
            nc.sync.dma_start(xt[:], ins["XT"])
            simT = [per.tile([128, _B], f32, name=f"simT{k}") for k in range(KT)]

            # ---------------- softmax(sim) and simT ----------------
            with tc.tile_pool(name="simtmp", bufs=1) as stp, \
                 tc.tile_pool(name="simpsum", bufs=2, space="PSUM") as sps, \
                 tc.tile_pool(name="trpsum", bufs=4, space="PSUM") as tps:
                eye = stp.tile([128, 128], f32, name="eye")
                nc.sync.dma_start(eye[:], ins["EYE"])
                ut = stp.tile([_D, _NU], f32, name="ut")
                nc.sync.dma_start(ut[:], ins["UT"])
                for t in range(RT):
                    lrow = stp.tile([128, _NU], f32, name="lrow", bufs=2)
                    for ucn in range(UC):
                        pl = sps.tile([128, UCW], f32, name="pl")
                        nc.tensor.matmul(pl[:], xt[:, t * 128:(t + 1) * 128],
                                         ut[:, ucn * UCW:(ucn + 1) * UCW],
                                         start=True, stop=True)
                        nc.scalar.activation(lrow[:, ucn * UCW:(ucn + 1) * UCW],
                                             pl[:],
                                             mybir.ActivationFunctionType.Copy,
                                             bias=0.0, scale=inv_scale)
                    mx = stp.tile([128, 1], f32, name="mx", bufs=2)
                    nc.vector.reduce_max(mx[:], lrow[:], axis=mybir.AxisListType.X)
                    nmx = stp.tile([128, 1], f32, name="nmx", bufs=2)
                    nc.scalar.mul(nmx[:], mx[:], -1.0)
                    erow = stp.tile([128, _NU], f32, name="erow", bufs=2)
                    zt = stp.tile([128, 1], f32, name="zt", bufs=2)
                    nc.scalar.activation(erow[:], lrow[:],
                                         mybir.ActivationFunctionType.Exp,
                                         bias=nmx[:], scale=1.0, accum_out=zt[:])
                    rz = stp.tile([128, 1], f32, name="rz", bufs=2)
                    nc.vector.reciprocal(rz[:], zt[:])
                    nc.vector.tensor_scalar_mul(erow[:], erow[:], rz[:])
                    # transpose blocks of [128, kw] into simT[k][:, t*128:...]
                    for k, (ko, kw) in enumerate(KTS):
                        pt = tps.tile([128, 128], f32, name="pt")
                        nc.tensor.transpose(pt[:kw, :], erow[:, ko:ko + kw],
                                            eye[:])
                        nc.scalar.copy(simT[k][:kw, t * 128:(t + 1) * 128],
                                       pt[:kw, :])

            # Veltkamp/Dekker split of simT: sh = 11-bit hi part (exact under
            # the PE's fp32r operand rounding), simT becomes the lo residue.
            sh = [per.tile([128, _B], f32, name=f"sh{k}") for k in range(KT)]
            with tc.tile_pool(name="vk", bufs=2) as vk:
                for k in range(KT):
                    tmp = vk.tile([128, _B], f32, name="tmp")
                    nc.vector.tensor_scalar_mul(tmp[:], simT[k][:], 4097.0)
                    d = vk.tile([128, _B], f32, name="d")
                    nc.vector.tensor_sub(d[:], tmp[:], simT[k][:])
                    nc.vector.tensor_sub(sh[k][:], tmp[:], d[:])
                    nc.vector.tensor_sub(simT[k][:], simT[k][:], sh[k][:])

            # ---------------- branch preds + extraction ----------------
            with tc.tile_pool(name="brtmp", bufs=1) as btp, \
                 tc.tile_pool(name="brpsum", bufs=2, space="PSUM") as bps:
                wsp = btp.tile([_D, _LAT], f32, name="wsp")
                nc.sync.dma_start(wsp[:], ins["Wsp"])
                wmp = btp.tile([_D, _LAT], f32, name="wmp")
                nc.sync.dma_start(wmp[:], ins["Wmp"])
                wsd = btp.tile([_LAT, _SSL], f32, name="wsd")
                nc.sync.dma_start(wsd[:], ins["Wsd"])
                wmd = btp.tile([_LAT, _MSL], f32, name="wmd")
                nc.sync.dma_start(wmd[:], ins["Wmd"])

                ast = btp.tile([_LAT, _B], f32, name="ast")
                amt = btp.tile([_LAT, _B], f32, name="amt")
                for half in range(2):
                    pa = bps.tile([_LAT, 256], f32, name="pa")
                    nc.tensor.matmul(pa[:], wsp[:],
                                     xt[:, half * 256:(half + 1) * 256],
                                     start=True, stop=True)
                    nc.scalar.copy(ast[:, half * 256:(half + 1) * 256], pa[:])
                    pb = bps.tile([_LAT, 256], f32, name="pb")
                    nc.tensor.matmul(pb[:], wmp[:],
                                     xt[:, half * 256:(half + 1) * 256],
                                     start=True, stop=True)
                    nc.scalar.copy(amt[:, half * 256:(half + 1) * 256], pb[:])

                for t in range(RT):
                    rsl = slice(t * 128, (t + 1) * 128)
                    for (nm, at_, wd, wmask, wsz, ov, oi) in (
                            ("s", ast, wsd, "MS", _SSL, "SV", "SI"),
                            ("m", amt, wmd, "MM", _MSL, "MV", "MI")):
                        pp = bps.tile([128, wsz], f32, name=f"pp{nm}")
                        nc.tensor.matmul(pp[:], at_[:, rsl], wd[:],
                                         start=True, stop=True)
                        pr = btp.tile([128, wsz], f32, name=f"pr{nm}", bufs=2)
                        msk = btp.tile([128, wsz], f32, name=f"msk{nm}", bufs=2)
                        nc.sync.dma_start(msk[:], ins[wmask][rsl, :])
                        nc.vector.tensor_mul(pr[:], pp[:], msk[:])
                        bv = btp.tile([128, _TK], f32, name=f"bv{nm}", bufs=2)
                        bi = btp.tile([128, _TK], u32, name=f"bi{nm}", bufs=2)
                        for r in range(5):
                            s8 = slice(8 * r, 8 * r + 8)
                            nc.vector.max(out=bv[:, s8], in_=pr[:])
                            nc.vector.max_index(out=bi[:, s8], in_max=bv[:, s8],
                                                in_values=pr[:])
                            nc.vector.match_replace(out=pr[:],
                                                    in_to_replace=bv[:, s8],
                                                    in_values=pr[:],
                                                    imm_value=-3.0e38)
                        nc.sync.dma_start(outs[ov][rsl, :], bv[:])
                        nc.sync.dma_start(outs[oi][rsl, :], bi[:])

            # ---------------- k_preds main matmul + chunked extraction ----------
            # per (rowtile, chunk): accumulate 16 k-tiles in PSUM, evict to a
            # transient SBUF tile, then 2 rounds of top-8 (16 candidates per
            # chunk; the global shard top-40 never has >16 in one 512-chunk).
            with tc.tile_pool(name="stream", bufs=1) as stream, \
                 tc.tile_pool(name="mainpsum", bufs=6, space="PSUM") as mps:
                cv = [stream.tile([128, _NCH * 16], f32, name=f"cv{t}")
                      for t in range(RT)]
                ci = [stream.tile([128, _NCH * 16], u32, name=f"ci{t}")
                      for t in range(RT)]
                for c in range(_NCH):
                    rh = [stream.tile([128, _CH], f32, name=f"rh{k}")
                          for k in range(KT)]
                    rl = [stream.tile([128, _CH], f32, name=f"rl{k}")
                          for k in range(KT)]
                    for k, (ko, kw) in enumerate(KTS):
                        csl = slice(c * _CH, (c + 1) * _CH)
                        nc.sync.dma_start(rh[k][:kw, :], ins["RH"][ko:ko + kw, csl])
                        nc.sync.dma_start(rl[k][:kw, :], ins["RL"][ko:ko + kw, csl])
                    for t in range(RT):
                        pk = mps.tile([128, _CH], f32, name="pk")
                        tsl = slice(t * 128, (t + 1) * 128)
                        for k, (ko, kw) in enumerate(KTS):
                            # kp = sum_k (sh+lo).T @ (rh+rl), dropping lo.T@rl
                            # (≈2^-24 relative). All operands are 11-13 bit
                            # mantissas, exact under fp32r operand rounding.
                            nc.tensor.matmul(pk[:],
                                             sh[k][:kw, tsl].bitcast(f32r),
                                             rh[k][:kw, :].bitcast(f32r),
                                             start=(k == 0), stop=False)
                            nc.tensor.matmul(pk[:],
                                             sh[k][:kw, tsl].bitcast(f32r),
                                             rl[k][:kw, :].bitcast(f32r),
                                             start=False, stop=False)
                            nc.tensor.matmul(pk[:],
                                             simT[k][:kw, tsl].bitcast(f32r),
                                             rh[k][:kw, :].bitcast(f32r),
                                             start=False, stop=(k == KT - 1))
                        kpc = stream.tile([128, _CH], f32, name="kpc", bufs=6)
                        nc.scalar.copy(kpc[:], pk[:])
                        c0 = slice(c * 16, c * 16 + 8)
                        c1 = slice(c * 16 + 8, c * 16 + 16)
                        nc.vector.max(out=cv[t][:, c0], in_=kpc[:])
                        nc.vector.max_index(out=ci[t][:, c0], in_max=cv[t][:, c0],
                                            in_values=kpc[:])
                        nc.vector.match_replace(out=kpc[:],
                                                in_to_replace=cv[t][:, c0],
                                                in_values=kpc[:],
                                                imm_value=-3.0e38)
                        nc.vector.max(out=cv[t][:, c1], in_=kpc[:])
                        nc.vector.max_index(out=ci[t][:, c1], in_max=cv[t][:, c1],
                                            in_values=kpc[:])
                for t in range(RT):
                    rsl = slice(t * 128, (t + 1) * 128)
                    nc.sync.dma_start(outs["KV"][rsl, :], cv[t][:])
                    nc.sync.dma_start(outs["KI"][rsl, :], ci[t][:])

    nc.compile()
    return nc


def _prep_inputs(X, mask, W_sprior, W_sdec, W_mprior, W_mdec,
                 user_ratings, user_personalities, top_map, mid_map):
    """Build the 8 per-core input maps (all float32 numpy)."""
    X = np.ascontiguousarray(X, dtype=np.float32)
    XT = np.ascontiguousarray(X.T)
    UT = np.ascontiguousarray(np.asarray(user_personalities, dtype=np.float32).T)
    eye = np.eye(128, dtype=np.float32)
    ur = np.asarray(user_ratings, dtype=np.float32)
    mask = np.asarray(mask, dtype=np.float32)
    Wsd_full = np.asarray(W_sdec, dtype=np.float32)
    Wmd_full = np.asarray(W_mdec, dtype=np.float32)
    mask_s = mask[:, np.asarray(top_map)]          # [B, 500]
    mask_m = mask[:, np.asarray(mid_map)]          # [B, 2000]

    C = np.float32(4097.0)
    t_ = ur * C
    rhi_full = t_ - (t_ - ur)          # 11-bit-mantissa hi part, f32-exact
    rlo_full = ur - rhi_full
    del t_

    in_maps = []
    for c in range(_NC):
        RH = np.zeros((_NU, _SHP), dtype=np.float32)
        RH[:, :_SHW] = rhi_full[:, c * _SHW:(c + 1) * _SHW]
        RL = np.zeros((_NU, _SHP), dtype=np.float32)
        RL[:, :_SHW] = rlo_full[:, c * _SHW:(c + 1) * _SHW]
        Wsd = np.zeros((_LAT, _SSL), dtype=np.float32)
        s0, s1 = c * _SSL, min((c + 1) * _SSL, _NS)
        if s0 < _NS:
            Wsd[:, :s1 - s0] = Wsd_full[:, s0:s1]
        Wmd = np.zeros((_LAT, _MSL), dtype=np.float32)
        m0, m1 = c * _MSL, min((c + 1) * _MSL, _NM)
        if m0 < _NM:
            Wmd[:, :m1 - m0] = Wmd_full[:, m0:m1]
        MS = np.zeros((_B, _SSL), dtype=np.float32)
        if s0 < _NS:
            MS[:, :s1 - s0] = mask_s[:, s0:s1]
        MM = np.zeros((_B, _MSL), dtype=np.float32)
        if m0 < _NM:
            MM[:, :m1 - m0] = mask_m[:, m0:m1]
        in_maps.append({
            "XT": XT, "UT": UT, "RH": RH, "RL": RL,
            "Wsp": np.asarray(W_sprior, dtype=np.float32),
            "Wmp": np.asarray(W_mprior, dtype=np.float32),
            "Wsd": Wsd, "Wmd": Wmd, "MS": MS, "MM": MM, "EYE": eye,
        })
    return in_maps


def _branch_topk(vals, gidx, valid, take):
    """Per-row: among valid candidates, top-`take` by (value desc, index asc).
    vals [B, n] f32, gidx [B, n] int64. Returns vals, gidx, ok each [B, take]."""
    v = np.where(valid, vals, np.float32(-np.inf))
    order = np.lexsort((gidx, -v.astype(np.float64)), axis=-1)
    v_s = np.take_along_axis(v, order, axis=1)[:, :take]
    g_s = np.take_along_axis(gidx, order, axis=1)[:, :take]
    ok = np.isfinite(v_s)
    return v_s.astype(np.float32), g_s, ok


def _merge(res, probs, top_map, mid_map):
    """Reproduce the reference fused scatter-add + top-20 from per-core
    candidate lists, in float32 with the reference's add order."""
    B = _B

    def gather(vname, iname, stride, nvalid):
        vals = np.concatenate([res[c][vname] for c in range(_NC)], axis=1)
        loc = np.concatenate([res[c][iname].astype(np.int64) for c in range(_NC)],
                             axis=1)
        base = np.concatenate([np.full((B, _TK), c * stride, np.int64)
                               for c in range(_NC)], axis=1)
        gidx = base + loc
        # pad columns carry exact zeros; the reference's zero/negative entries
        # contribute nothing to fused, so val>0 is the candidate filter.
        valid = (vals > 0) & (loc < stride) & (gidx < nvalid)
        return vals, gidx, valid

    sv, sg, s_ok = gather("SV", "SI", _SSL, _NS)
    sg_cat = np.where(s_ok, top_map[np.clip(sg, 0, _NS - 1)], 0)
    mv, mg, m_ok = gather("MV", "MI", _MSL, _NM)
    mg_cat = np.where(m_ok, mid_map[np.clip(mg, 0, _NM - 1)], 0)
    # k-branch: per-chunk candidates; local idx = chunk*512 + within-chunk idx
    kvals = np.concatenate([res[c]["KV"] for c in range(_NC)], axis=1)
    kloc = np.concatenate([res[c]["KI"].astype(np.int64) for c in range(_NC)],
                          axis=1)
    chunk_of = np.tile(np.repeat(np.arange(_NCH, dtype=np.int64), 16), _NC)
    core_of = np.repeat(np.arange(_NC, dtype=np.int64), _NCH * 16)
    kg = core_of[None, :] * _SHW + chunk_of[None, :] * _CH + kloc
    k_ok = (kvals > 0) & (chunk_of[None, :] * _CH + kloc < _SHW) & (kg < _NI)
    kv, k_ok = kvals, k_ok

    sv40, sg40, sok40 = _branch_topk(sv, sg_cat, s_ok, _TK)
    mv40, mg40, mok40 = _branch_topk(mv, mg_cat, m_ok, _TK)
    kv40, kg40, kok40 = _branch_topk(kv, kg, k_ok, _TK)

    # contributions in the reference's add order: s (probs0), m (probs1), k (probs2)
    c_s = np.where(sok40, (sv40 * probs[:, 0:1]).astype(np.float32), np.float32(0))
    c_m = np.where(mok40, (mv40 * probs[:, 1:2]).astype(np.float32), np.float32(0))
    c_k = np.where(kok40, (kv40 * probs[:, 2:3]).astype(np.float32), np.float32(0))

    idx = np.concatenate([sg40, mg40, kg40], axis=1)              # [B, 120]
    con = np.concatenate([c_s, c_m, c_k], axis=1).astype(np.float32)
    ok = np.concatenate([sok40, mok40, kok40], axis=1)
    brk = np.concatenate([np.full((B, _TK), i, np.int64) for i in range(3)], axis=1)

    idx = np.where(ok, idx, np.int64(_NI + 1))                    # park invalid
    order = np.lexsort((brk, idx), axis=-1)
    idx_s = np.take_along_axis(idx, order, axis=1)
    con_s = np.take_along_axis(con, order, axis=1)
    ok_s = np.take_along_axis(ok, order, axis=1)

    # sequential f32 adds within runs of equal idx (run length <= 3, ordered
    # s -> m -> k by the brk tiebreaker, matching the reference)
    n = idx_s.shape[1]
    first = np.ones(idx_s.shape, dtype=bool)
    first[:, 1:] = idx_s[:, 1:] != idx_s[:, :-1]
    vals_acc = np.zeros((B, n), dtype=np.float32)
    cur = np.zeros(B, dtype=np.float32)
    for j in range(n):
        cur = np.where(first[:, j], con_s[:, j],
                       (cur + con_s[:, j]).astype(np.float32)).astype(np.float32)
        vals_acc[:, j] = cur
    last = np.ones(idx_s.shape, dtype=bool)
    last[:, :-1] = first[:, 1:]
    fuse_val = np.where(last & ok_s, vals_acc, np.float32(-np.inf))
    fuse_idx = np.where(last & ok_s, idx_s, np.int64(_NI + 1))

    order2 = np.lexsort((fuse_idx, -fuse_val.astype(np.float64)), axis=-1)
    top = np.take_along_axis(fuse_idx, order2, axis=1)[:, :_K]
    return top.astype(np.int32)


def kernel(X, mask, W_sprior, W_sdec, W_mprior, W_mdec, W_mapper,
           user_ratings, user_personalities, top_map, mid_map, k,
           _want_trace=False):
    from concourse.bass_utils import run_bass_kernel_spmd

    assert int(k) == _K
    if "nc" not in _cache:
        _cache["nc"] = _build_program()
    nc = _cache["nc"]

    in_maps = _prep_inputs(X, mask, W_sprior, W_sdec, W_mprior, W_mdec,
                           user_ratings, user_personalities, top_map, mid_map)
    kw = {}
    if _want_trace:
        kw = dict(trace=True)
    rr = run_bass_kernel_spmd(nc, in_maps, core_ids=list(range(_NC)), **kw)
    res = rr.results

    # probs = softmax(X @ W_mapper) in f32, matching the reference's op order
    Xf = np.asarray(X, dtype=np.float32)
    pl = Xf @ np.asarray(W_mapper, dtype=np.float32)
    pl = pl - pl.max(axis=1, keepdims=True)
    pe = np.exp(pl)
    probs = (pe / pe.sum(axis=1, keepdims=True)).astype(np.float32)

    out = _merge(res, probs, np.asarray(top_map).astype(np.int64),
                 np.asarray(mid_map).astype(np.int64))
    if _want_trace:
        return out, rr
    return out


# revision 7
# speedup vs baseline: 1.2203x; 1.2203x over previous
"""Trainium2 Bass kernel for nn_EnsembleModel (ensemble recommender).

Contract: kernel(**inputs) takes FULL unsharded inputs (as produced by the
reference setup_inputs) and returns the FULL [512, 20] int32 output.

Strategy (8 NeuronCores, SPMD — identical program, per-core data):
  - items sharded 8x: each core owns 6250 catalog columns of user_ratings
    (padded to 6656 = 13*512) and computes k_preds = softmax(X@U.T/sqrt(32)) @ R
    for its shard with exact-fp32 PE matmuls, then extracts per-row top-40
    (values + indices) with max8/max_index/match_replace.
  - the two decoder branches are column-sharded 8x as well (64 / 256 cols per
    core); each core computes its slice of preds = (X@W_prior)@W_dec_slice,
    applies the gathered mask columns, and extracts per-row top-40.
  - host merges the per-core candidate lists (provably lossless: a shard-local
    top-40 always contains the shard's contribution to the global top-40) and
    reproduces the reference's fused scatter-add + final top-20 in float32.
"""

import numpy as np

_B, _D, _LAT = 512, 32, 128
_NS, _NM, _NI, _NU = 500, 2000, 50000, 2000
_NC = 8
_SHW = _NI // _NC            # 6250 items per core
_CH = 512
_NCH = 13
_SHP = _CH * _NCH            # 6656 padded shard width
_SSL = 64                    # s-branch cols per core  (8*64  >= 500)
_MSL = 256                   # m-branch cols per core  (8*256 >= 2000)
_TK = 40                     # two_k
_K = 20

_cache = {}


def _build_program():
    import concourse.bacc as bacc
    import concourse.tile as tile
    from concourse import mybir

    nc = bacc.Bacc("TRN2", target_bir_lowering=False, debug=False, num_devices=_NC)
    f32 = mybir.dt.float32
    f32r = mybir.dt.float32r
    u32 = mybir.dt.uint32

    ins = {}
    def inp(name, shape, dt=None):
        ins[name] = nc.dram_tensor(name, shape, dt or f32,
                                   kind="ExternalInput").ap()
    inp("XT", [_D, _B])            # X transposed (host-prepped)
    inp("UT", [_D, _NU])           # user_personalities transposed
    inp("RH", [_NU, _SHP], f32r)   # ratings shard hi (11-bit mantissa)
    inp("RL", [_NU, _SHP], f32r)   # ratings shard lo (R - RH, exact)
    inp("Wsp", [_D, _LAT])         # W_sprior
    inp("Wmp", [_D, _LAT])         # W_mprior
    inp("Wsd", [_LAT, _SSL])       # W_sdec column slice (zero-padded)
    inp("Wmd", [_LAT, _MSL])       # W_mdec column slice
    inp("MS", [_B, _SSL])          # mask cols for the s slice
    inp("MM", [_B, _MSL])          # mask cols for the m slice
    inp("EYE", [128, 128])         # identity for PE transpose

    outs = {}
    def outp(name, shape, dt):
        outs[name] = nc.dram_tensor(name, shape, dt, kind="ExternalOutput").ap()
    outp("KV", [_B, _NCH * 16], f32)
    outp("KI", [_B, _NCH * 16], u32)
    outp("SV", [_B, _TK], f32)
    outp("SI", [_B, _TK], u32)
    outp("MV", [_B, _TK], f32)
    outp("MI", [_B, _TK], u32)

    RT = 4                       # row tiles of 128
    UCW = 500                    # logits chunk width (4 * 500 = 2000)
    UC = _NU // UCW
    KTS = [(o, min(128, _NU - o)) for o in range(0, _NU, 128)]  # 15x128 + 1x80
    KT = len(KTS)
    inv_scale = float(np.float32(1.0) / np.float32(np.sqrt(np.float32(_D))))

    with tile.TileContext(nc) as tc:
        with tc.tile_pool(name="persist", bufs=1) as per:
            xt = per.tile([_D, _B], f32, name="xt")
            nc.sync.dma_start(xt[:], ins["XT"])
            simT = [per.tile([128, _B], f32, name=f"simT{k}") for k in range(KT)]

            # ---------------- softmax(sim) and simT ----------------
            with tc.tile_pool(name="simtmp", bufs=1) as stp, \
                 tc.tile_pool(name="simpsum", bufs=2, space="PSUM") as sps, \
                 tc.tile_pool(name="trpsum", bufs=4, space="PSUM") as tps:
                eye = stp.tile([128, 128], f32, name="eye")
                nc.sync.dma_start(eye[:], ins["EYE"])
                ut = stp.tile([_D, _NU], f32, name="ut")
                nc.sync.dma_start(ut[:], ins["UT"])
                for t in range(RT):
                    lrow = stp.tile([128, _NU], f32, name="lrow", bufs=2)
                    for ucn in range(UC):
                        pl = sps.tile([128, UCW], f32, name="pl")
                        nc.tensor.matmul(pl[:], xt[:, t * 128:(t + 1) * 128],
                                         ut[:, ucn * UCW:(ucn + 1) * UCW],
                                         start=True, stop=True)
                        nc.scalar.activation(lrow[:, ucn * UCW:(ucn + 1) * UCW],
                                             pl[:],
                                             mybir.ActivationFunctionType.Copy,
                                             bias=0.0, scale=inv_scale)
                    mx = stp.tile([128, 1], f32, name="mx", bufs=2)
                    nc.vector.reduce_max(mx[:], lrow[:], axis=mybir.AxisListType.X)
                    nmx = stp.tile([128, 1], f32, name="nmx", bufs=2)
                    nc.scalar.mul(nmx[:], mx[:], -1.0)
                    erow = stp.tile([128, _NU], f32, name="erow", bufs=2)
                    zt = stp.tile([128, 1], f32, name="zt", bufs=2)
                    nc.scalar.activation(erow[:], lrow[:],
                                         mybir.ActivationFunctionType.Exp,
                                         bias=nmx[:], scale=1.0, accum_out=zt[:])
                    rz = stp.tile([128, 1], f32, name="rz", bufs=2)
                    nc.vector.reciprocal(rz[:], zt[:])
                    nc.vector.tensor_scalar_mul(erow[:], erow[:], rz[:])
                    # transpose blocks of [128, kw] into simT[k][:, t*128:...]
                    for k, (ko, kw) in enumerate(KTS):
                        pt = tps.tile([128, 128], f32, name="pt")
                        nc.tensor.transpose(pt[:kw, :], erow[:, ko:ko + kw],
                                            eye[:])
                        nc.scalar.copy(simT[k][:kw, t * 128:(t + 1) * 128],
                                       pt[:kw, :])

            # Veltkamp/Dekker split of simT: sh = 11-bit hi part (exact under
            # the PE's fp32r operand rounding), simT becomes the lo residue.
            shr = [per.tile([128, _B], f32r, name=f"shr{k}") for k in range(KT)]
            slor = [per.tile([128, _B], f32r, name=f"slor{k}") for k in range(KT)]
            with tc.tile_pool(name="vk", bufs=2) as vk:
                for k in range(KT):
                    tmp = vk.tile([128, _B], f32, name="tmp")
                    nc.vector.tensor_scalar_mul(tmp[:], simT[k][:], 4097.0)
                    d = vk.tile([128, _B], f32, name="d")
                    nc.vector.tensor_sub(d[:], tmp[:], simT[k][:])
                    shf = vk.tile([128, _B], f32, name="shf")
                    nc.vector.tensor_sub(shf[:], tmp[:], d[:])
                    slof = vk.tile([128, _B], f32, name="slof")
                    nc.vector.tensor_sub(slof[:], simT[k][:], shf[:])
                    # dtype-cast copies so the BIR verifier sees fp32r
                    # producers for the fp32r matmuls (values are already
                    # 11-bit-rounded, so the cast is bit-preserving)
                    nc.scalar.copy(shr[k][:], shf[:])
                    nc.scalar.copy(slor[k][:], slof[:])

            # ---------------- branch preds + extraction ----------------
            with tc.tile_pool(name="brtmp", bufs=1) as btp, \
                 tc.tile_pool(name="brpsum", bufs=2, space="PSUM") as bps:
                wsp = btp.tile([_D, _LAT], f32, name="wsp")
                nc.sync.dma_start(wsp[:], ins["Wsp"])
                wmp = btp.tile([_D, _LAT], f32, name="wmp")
                nc.sync.dma_start(wmp[:], ins["Wmp"])
                wsd = btp.tile([_LAT, _SSL], f32, name="wsd")
                nc.sync.dma_start(wsd[:], ins["Wsd"])
                wmd = btp.tile([_LAT, _MSL], f32, name="wmd")
                nc.sync.dma_start(wmd[:], ins["Wmd"])

                ast = btp.tile([_LAT, _B], f32, name="ast")
                amt = btp.tile([_LAT, _B], f32, name="amt")
                for half in range(2):
                    pa = bps.tile([_LAT, 256], f32, name="pa")
                    nc.tensor.matmul(pa[:], wsp[:],
                                     xt[:, half * 256:(half + 1) * 256],
                                     start=True, stop=True)
                    nc.scalar.copy(ast[:, half * 256:(half + 1) * 256], pa[:])
                    pb = bps.tile([_LAT, 256], f32, name="pb")
                    nc.tensor.matmul(pb[:], wmp[:],
                                     xt[:, half * 256:(half + 1) * 256],
                                     start=True, stop=True)
                    nc.scalar.copy(amt[:, half * 256:(half + 1) * 256], pb[:])

                for t in range(RT):
                    rsl = slice(t * 128, (t + 1) * 128)
                    for (nm, at_, wd, wmask, wsz, ov, oi) in (
                            ("s", ast, wsd, "MS", _SSL, "SV", "SI"),
                            ("m", amt, wmd, "MM", _MSL, "MV", "MI")):
                        pp = bps.tile([128, wsz], f32, name=f"pp{nm}")
                        nc.tensor.matmul(pp[:], at_[:, rsl], wd[:],
                                         start=True, stop=True)
                        pr = btp.tile([128, wsz], f32, name=f"pr{nm}", bufs=2)
                        msk = btp.tile([128, wsz], f32, name=f"msk{nm}", bufs=2)
                        nc.sync.dma_start(msk[:], ins[wmask][rsl, :])
                        nc.vector.tensor_mul(pr[:], pp[:], msk[:])
                        bv = btp.tile([128, _TK], f32, name=f"bv{nm}", bufs=2)
                        bi = btp.tile([128, _TK], u32, name=f"bi{nm}", bufs=2)
                        for r in range(5):
                            s8 = slice(8 * r, 8 * r + 8)
                            nc.vector.max(out=bv[:, s8], in_=pr[:])
                            nc.vector.max_index(out=bi[:, s8], in_max=bv[:, s8],
                                                in_values=pr[:])
                            nc.vector.match_replace(out=pr[:],
                                                    in_to_replace=bv[:, s8],
                                                    in_values=pr[:],
                                                    imm_value=-3.0e38)
                        nc.sync.dma_start(outs[ov][rsl, :], bv[:])
                        nc.sync.dma_start(outs[oi][rsl, :], bi[:])

            # ---------------- k_preds main matmul + chunked extraction ----------
            # per (rowtile, chunk): accumulate 16 k-tiles in PSUM, evict to a
            # transient SBUF tile, then 2 rounds of top-8 (16 candidates per
            # chunk; the global shard top-40 never has >16 in one 512-chunk).
            with tc.tile_pool(name="stream", bufs=1) as stream, \
                 tc.tile_pool(name="mainpsum", bufs=6, space="PSUM") as mps:
                cv = [stream.tile([128, _NCH * 16], f32, name=f"cv{t}")
                      for t in range(RT)]
                ci = [stream.tile([128, _NCH * 16], u32, name=f"ci{t}")
                      for t in range(RT)]
                for c in range(_NCH):
                    rh = [stream.tile([128, _CH], f32r, name=f"rh{k}")
                          for k in range(KT)]
                    rl = [stream.tile([128, _CH], f32r, name=f"rl{k}")
                          for k in range(KT)]
                    for k, (ko, kw) in enumerate(KTS):
                        csl = slice(c * _CH, (c + 1) * _CH)
                        nc.sync.dma_start(rh[k][:kw, :], ins["RH"][ko:ko + kw, csl])
                        nc.sync.dma_start(rl[k][:kw, :], ins["RL"][ko:ko + kw, csl])
                    for t in range(RT):
                        pk = mps.tile([128, _CH], f32, name="pk")
                        tsl = slice(t * 128, (t + 1) * 128)
                        for k, (ko, kw) in enumerate(KTS):
                            # kp = sum_k (sh+lo).T @ (rh+rl), dropping lo.T@rl
                            # (≈2^-24 relative). All operands are 11-13 bit
                            # mantissas, exact under fp32r operand rounding.
                            nc.tensor.matmul(pk[:],
                                             shr[k][:kw, tsl],
                                             rh[k][:kw, :],
                                             start=(k == 0), stop=False)
                            nc.tensor.matmul(pk[:],
                                             shr[k][:kw, tsl],
                                             rl[k][:kw, :],
                                             start=False, stop=False)
                            nc.tensor.matmul(pk[:],
                                             slor[k][:kw, tsl],
                                             rh[k][:kw, :],
                                             start=False, stop=(k == KT - 1))
                        kpc = stream.tile([128, _CH], f32, name="kpc", bufs=6)
                        nc.scalar.copy(kpc[:], pk[:])
                        c0 = slice(c * 16, c * 16 + 8)
                        c1 = slice(c * 16 + 8, c * 16 + 16)
                        nc.vector.max(out=cv[t][:, c0], in_=kpc[:])
                        nc.vector.max_index(out=ci[t][:, c0], in_max=cv[t][:, c0],
                                            in_values=kpc[:])
                        nc.vector.match_replace(out=kpc[:],
                                                in_to_replace=cv[t][:, c0],
                                                in_values=kpc[:],
                                                imm_value=-3.0e38)
                        nc.vector.max(out=cv[t][:, c1], in_=kpc[:])
                        nc.vector.max_index(out=ci[t][:, c1], in_max=cv[t][:, c1],
                                            in_values=kpc[:])
                for t in range(RT):
                    rsl = slice(t * 128, (t + 1) * 128)
                    nc.sync.dma_start(outs["KV"][rsl, :], cv[t][:])
                    nc.sync.dma_start(outs["KI"][rsl, :], ci[t][:])

    nc.compile()
    return nc


def _prep_inputs(X, mask, W_sprior, W_sdec, W_mprior, W_mdec,
                 user_ratings, user_personalities, top_map, mid_map):
    """Build the 8 per-core input maps (all float32 numpy)."""
    X = np.ascontiguousarray(X, dtype=np.float32)
    XT = np.ascontiguousarray(X.T)
    UT = np.ascontiguousarray(np.asarray(user_personalities, dtype=np.float32).T)
    eye = np.eye(128, dtype=np.float32)
    ur = np.asarray(user_ratings, dtype=np.float32)
    mask = np.asarray(mask, dtype=np.float32)
    Wsd_full = np.asarray(W_sdec, dtype=np.float32)
    Wmd_full = np.asarray(W_mdec, dtype=np.float32)
    mask_s = mask[:, np.asarray(top_map)]          # [B, 500]
    mask_m = mask[:, np.asarray(mid_map)]          # [B, 2000]

    C = np.float32(4097.0)
    t_ = ur * C
    rhi_full = t_ - (t_ - ur)          # 11-bit-mantissa hi part, f32-exact
    rlo_full = ur - rhi_full
    del t_

    in_maps = []
    for c in range(_NC):
        RH = np.zeros((_NU, _SHP), dtype=np.float32)
        RH[:, :_SHW] = rhi_full[:, c * _SHW:(c + 1) * _SHW]
        RL = np.zeros((_NU, _SHP), dtype=np.float32)
        RL[:, :_SHW] = rlo_full[:, c * _SHW:(c + 1) * _SHW]
        Wsd = np.zeros((_LAT, _SSL), dtype=np.float32)
        s0, s1 = c * _SSL, min((c + 1) * _SSL, _NS)
        if s0 < _NS:
            Wsd[:, :s1 - s0] = Wsd_full[:, s0:s1]
        Wmd = np.zeros((_LAT, _MSL), dtype=np.float32)
        m0, m1 = c * _MSL, min((c + 1) * _MSL, _NM)
        if m0 < _NM:
            Wmd[:, :m1 - m0] = Wmd_full[:, m0:m1]
        MS = np.zeros((_B, _SSL), dtype=np.float32)
        if s0 < _NS:
            MS[:, :s1 - s0] = mask_s[:, s0:s1]
        MM = np.zeros((_B, _MSL), dtype=np.float32)
        if m0 < _NM:
            MM[:, :m1 - m0] = mask_m[:, m0:m1]
        in_maps.append({
            "XT": XT, "UT": UT, "RH": RH, "RL": RL,
            "Wsp": np.asarray(W_sprior, dtype=np.float32),
            "Wmp": np.asarray(W_mprior, dtype=np.float32),
            "Wsd": Wsd, "Wmd": Wmd, "MS": MS, "MM": MM, "EYE": eye,
        })
    return in_maps


def _branch_topk(vals, gidx, valid, take):
    """Per-row: among valid candidates, top-`take` by (value desc, index asc).
    vals [B, n] f32, gidx [B, n] int64. Returns vals, gidx, ok each [B, take]."""
    v = np.where(valid, vals, np.float32(-np.inf))
    order = np.lexsort((gidx, -v.astype(np.float64)), axis=-1)
    v_s = np.take_along_axis(v, order, axis=1)[:, :take]
    g_s = np.take_along_axis(gidx, order, axis=1)[:, :take]
    ok = np.isfinite(v_s)
    return v_s.astype(np.float32), g_s, ok


def _merge(res, probs, top_map, mid_map):
    """Reproduce the reference fused scatter-add + top-20 from per-core
    candidate lists, in float32 with the reference's add order."""
    B = _B

    def gather(vname, iname, stride, nvalid):
        vals = np.concatenate([res[c][vname] for c in range(_NC)], axis=1)
        loc = np.concatenate([res[c][iname].astype(np.int64) for c in range(_NC)],
                             axis=1)
        base = np.concatenate([np.full((B, _TK), c * stride, np.int64)
                               for c in range(_NC)], axis=1)
        gidx = base + loc
        # pad columns carry exact zeros; the reference's zero/negative entries
        # contribute nothing to fused, so val>0 is the candidate filter.
        valid = (vals > 0) & (loc < stride) & (gidx < nvalid)
        return vals, gidx, valid

    sv, sg, s_ok = gather("SV", "SI", _SSL, _NS)
    sg_cat = np.where(s_ok, top_map[np.clip(sg, 0, _NS - 1)], 0)
    mv, mg, m_ok = gather("MV", "MI", _MSL, _NM)
    mg_cat = np.where(m_ok, mid_map[np.clip(mg, 0, _NM - 1)], 0)
    # k-branch: per-chunk candidates; local idx = chunk*512 + within-chunk idx
    kvals = np.concatenate([res[c]["KV"] for c in range(_NC)], axis=1)
    kloc = np.concatenate([res[c]["KI"].astype(np.int64) for c in range(_NC)],
                          axis=1)
    chunk_of = np.tile(np.repeat(np.arange(_NCH, dtype=np.int64), 16), _NC)
    core_of = np.repeat(np.arange(_NC, dtype=np.int64), _NCH * 16)
    kg = core_of[None, :] * _SHW + chunk_of[None, :] * _CH + kloc
    k_ok = (kvals > 0) & (chunk_of[None, :] * _CH + kloc < _SHW) & (kg < _NI)
    kv, k_ok = kvals, k_ok

    sv40, sg40, sok40 = _branch_topk(sv, sg_cat, s_ok, _TK)
    mv40, mg40, mok40 = _branch_topk(mv, mg_cat, m_ok, _TK)
    kv40, kg40, kok40 = _branch_topk(kv, kg, k_ok, _TK)

    # contributions in the reference's add order: s (probs0), m (probs1), k (probs2)
    c_s = np.where(sok40, (sv40 * probs[:, 0:1]).astype(np.float32), np.float32(0))
    c_m = np.where(mok40, (mv40 * probs[:, 1:2]).astype(np.float32), np.float32(0))
    c_k = np.where(kok40, (kv40 * probs[:, 2:3]).astype(np.float32), np.float32(0))

    idx = np.concatenate([sg40, mg40, kg40], axis=1)              # [B, 120]
    con = np.concatenate([c_s, c_m, c_k], axis=1).astype(np.float32)
    ok = np.concatenate([sok40, mok40, kok40], axis=1)
    brk = np.concatenate([np.full((B, _TK), i, np.int64) for i in range(3)], axis=1)

    idx = np.where(ok, idx, np.int64(_NI + 1))                    # park invalid
    order = np.lexsort((brk, idx), axis=-1)
    idx_s = np.take_along_axis(idx, order, axis=1)
    con_s = np.take_along_axis(con, order, axis=1)
    ok_s = np.take_along_axis(ok, order, axis=1)

    # sequential f32 adds within runs of equal idx (run length <= 3, ordered
    # s -> m -> k by the brk tiebreaker, matching the reference)
    n = idx_s.shape[1]
    first = np.ones(idx_s.shape, dtype=bool)
    first[:, 1:] = idx_s[:, 1:] != idx_s[:, :-1]
    vals_acc = np.zeros((B, n), dtype=np.float32)
    cur = np.zeros(B, dtype=np.float32)
    for j in range(n):
        cur = np.where(first[:, j], con_s[:, j],
                       (cur + con_s[:, j]).astype(np.float32)).astype(np.float32)
        vals_acc[:, j] = cur
    last = np.ones(idx_s.shape, dtype=bool)
    last[:, :-1] = first[:, 1:]
    fuse_val = np.where(last & ok_s, vals_acc, np.float32(-np.inf))
    fuse_idx = np.where(last & ok_s, idx_s, np.int64(_NI + 1))

    order2 = np.lexsort((fuse_idx, -fuse_val.astype(np.float64)), axis=-1)
    top = np.take_along_axis(fuse_idx, order2, axis=1)[:, :_K]
    return top.astype(np.int32)


def kernel(X, mask, W_sprior, W_sdec, W_mprior, W_mdec, W_mapper,
           user_ratings, user_personalities, top_map, mid_map, k,
           _want_trace=False):
    from concourse.bass_utils import run_bass_kernel_spmd

    assert int(k) == _K
    if "nc" not in _cache:
        _cache["nc"] = _build_program()
    nc = _cache["nc"]

    in_maps = _prep_inputs(X, mask, W_sprior, W_sdec, W_mprior, W_mdec,
                           user_ratings, user_personalities, top_map, mid_map)
    kw = {}
    if _want_trace:
        kw = dict(trace=True)
    rr = run_bass_kernel_spmd(nc, in_maps, core_ids=list(range(_NC)), **kw)
    res = rr.results

    # probs = softmax(X @ W_mapper) in f32, matching the reference's op order
    Xf = np.asarray(X, dtype=np.float32)
    pl = Xf @ np.asarray(W_mapper, dtype=np.float32)
    pl = pl - pl.max(axis=1, keepdims=True)
    pe = np.exp(pl)
    probs = (pe / pe.sum(axis=1, keepdims=True)).astype(np.float32)

    out = _merge(res, probs, np.asarray(top_map).astype(np.int64),
                 np.asarray(mid_map).astype(np.int64))
    if _want_trace:
        return out, rr
    return out
